# revision 12
# baseline (speedup 1.0000x reference)
"""Trainium2 Bass kernel for a 2-layer GraphSAGE classifier (BGNNClassifier).

Reference computation (see problem):
    h1 = relu(mean_agg(x) @ W1l.T + b1 + x @ W1r.T)
    h2 = relu(mean_agg(h1) @ W2l.T + b2 + h1 @ W2r.T)
    pooled = segment_mean(h2, batch)          # [G, H]
    out = log_softmax(pooled @ fcW.T + fcb)   # [G, O]

Distribution strategy (8 NeuronCores, SPMD, one NEFF):
  - Nodes are partitioned into 8 contiguous shards (12500 each); each core
    computes h1/h2 for its own nodes.
  - x (and later h1) is replicated in each core's DRAM; per-edge neighbor
    features are fetched with the dma_gather SWDGE instruction (MoE-style
    row gather).  dma_gather indices are int16, so the gather table is
    split into 4 chunks of 32768 rows; edges are grouped by (dst-tile,
    src-chunk "phase").
  - Mean aggregation is computed on the TensorEngine as a sequence of
    one-hot matmuls: for each 128-edge chunk, aggrT[:, tile] +=
    G_chunk.T @ onehot(dstrel) * recip(deg).  The one-hot matrices are
    built on DVE/ACT from per-chunk metadata (iota == dstrel) * recip.
  - After layer 1, each core's h1 shard is AllGather'd so layer-2 gathers
    can read any neighbor.
  - Per-graph pooling is another one-hot matmul accumulated over all
    tiles; partial per-graph sums are AllReduce'd, then every core
    computes the (identical) logits + log_softmax.  Core 0's output is
    returned.

All graph-structure-dependent index/metadata arrays (gather indices,
one-hot metadata, degree reciprocals, graph ids) are prepared on the host
as *data*; the instruction stream is identical across cores (SPMD).
"""

import numpy as np

# ---------------------------------------------------------------------------
# Problem configuration (hardcoded for the full problem; overridable for dev)
# ---------------------------------------------------------------------------
CFG = dict(
    N=100000,      # nodes
    E=1600000,     # edges
    D=64,          # in features
    H=64,          # hidden
    O=16,          # classes
    G=512,         # graphs
    NCORES=8,
    NPHASE=4,      # number of gather-table chunks (int16 index range each)
    CPP=4,         # chunks per (tile, phase): 512 edge slots
    PACK_LIMIT=500,  # per-(tile,phase) edge budget while packing
    B=4,           # tiles per gather block
    P=128,
)

_BUILD_CACHE = {}


# ---------------------------------------------------------------------------
# Host-side preprocessing
# ---------------------------------------------------------------------------

def _pack_tiles(counts, pack_limit, P=128):
    """Pack nodes (in order) into tiles s.t. every per-tile counter sum
    <= pack_limit and node count <= P.  counts: [n_nodes, K] int64.
    Returns list of tile start indices (len T+1, last == n_nodes)."""
    n = counts.shape[0]
    cum = np.concatenate([np.zeros((1, counts.shape[1]), np.int64),
                          np.cumsum(counts, axis=0)], axis=0)  # [n+1, K]
    starts = [0]
    s = 0
    while s < n:
        e_lim = min(n, s + P)
        # max e with cum[e, k] - cum[s, k] <= pack_limit for all k
        e = e_lim
        for k in range(counts.shape[1]):
            ek = int(np.searchsorted(cum[:, k], cum[s, k] + pack_limit,
                                     side="right")) - 1
            e = min(e, ek)
        if e <= s:
            raise ValueError(
                f"node {s} alone exceeds pack limit (deg counts {counts[s]})")
        starts.append(e)
        s = e
    return starts


def _rank_within_groups(key, n_groups):
    """For int array key, return rank of each element within its key-group
    (stable order)."""
    order = np.argsort(key, kind="stable")
    sk = key[order]
    # start offset of each group in sorted order
    group_sizes = np.bincount(sk, minlength=n_groups)
    group_starts = np.concatenate([[0], np.cumsum(group_sizes)[:-1]])
    ranks_sorted = np.arange(len(key)) - group_starts[sk]
    ranks = np.empty(len(key), np.int64)
    ranks[order] = ranks_sorted
    return ranks


def _wrap_idx(idx_call):
    """dma_gather index layout: idx i -> [16r + i%16, i//16], replicated
    for the 8 Q7 cores.  idx_call: [n] int -> [128, n//16] int16."""
    n = idx_call.shape[0]
    assert n % 16 == 0
    w = idx_call.reshape(n // 16, 16).T.astype(np.int16)   # [16, n//16]
    return np.tile(w, (8, 1))                              # [128, n//16]


def preprocess(x, W1l, b1, W1r, W2l, b2, W2r, fcW, fcb, edge_index, batch,
               cfg=CFG):
    """Builds per-core input maps + layout info. Returns (in_maps, info)."""
    N, E, D, H, O, G = (cfg["N"], cfg["E"], cfg["D"], cfg["H"], cfg["O"],
                        cfg["G"])
    NC, NPH, CPP, P, B = (cfg["NCORES"], cfg["NPHASE"],
                          cfg["CPP"], cfg["P"], cfg["B"])
    SLOTS_PP = CPP * P            # edge slots per (tile, phase)
    NPC = N // NC
    PACK_LIMIT = cfg["PACK_LIMIT"]
    CHUNK1 = -(-N // NPH)
    assert CHUNK1 <= 32767

    x = np.asarray(x, np.float32)
    src = np.asarray(edge_index[0], np.int64)
    dst = np.asarray(edge_index[1], np.int64)
    batch = np.asarray(batch, np.int64)

    deg = np.bincount(dst, minlength=N)
    recip = (1.0 / np.maximum(deg, 1)).astype(np.float32)
    gsize = np.bincount(batch, minlength=G)
    grecip_g = (1.0 / np.maximum(gsize, 1)).astype(np.float32)

    core_of = dst // NPC
    phase1 = src // CHUNK1                     # L1 phase per edge

    # --- iterative packing (L2 phases depend on packed positions) --------
    cnt1 = np.bincount(dst * NPH + phase1, minlength=N * NPH) \
             .reshape(N, NPH)

    phase2 = phase1.copy()      # initial guess
    starts_per_core = None
    T = None
    CHUNK2 = None
    for _ in range(8):
        cnt2 = np.bincount(dst * NPH + phase2, minlength=N * NPH) \
                 .reshape(N, NPH)
        counts8 = np.concatenate([cnt1, cnt2], axis=1)    # [N, 8]
        starts_per_core = []
        for k in range(NC):
            starts_per_core.append(
                _pack_tiles(counts8[k * NPC:(k + 1) * NPC], PACK_LIMIT, P))
        T_new = max(len(s) - 1 for s in starts_per_core)
        T_new = -(-T_new // B) * B            # round up to block multiple
        # node -> (tile, slot) and global h1 row
        t_of = np.empty(N, np.int64)
        q_of = np.empty(N, np.int64)
        for k in range(NC):
            st = np.asarray(starts_per_core[k])
            u = np.arange(NPC)
            tk = np.searchsorted(st, u, side="right") - 1
            t_of[k * NPC:(k + 1) * NPC] = tk
            q_of[k * NPC:(k + 1) * NPC] = u - st[tk]
        h1row_node = (np.arange(N) // NPC) * (T_new * P) + t_of * P + q_of
        CHUNK2_new = -(-(NC * T_new * P) // NPH)
        assert CHUNK2_new <= 32767, f"T={T_new} too large for int16 L2 chunks"
        phase2_new = h1row_node[src] // CHUNK2_new
        if T == T_new and np.array_equal(phase2_new, phase2):
            CHUNK2 = CHUNK2_new
            break
        phase2 = phase2_new
        T = T_new
        CHUNK2 = CHUNK2_new
    T = T_new
    NB = T // B

    # verify per-(tile,phase) budgets hold for the FINAL packing
    tile_of_dst = t_of[dst]
    seg1 = (core_of * T + tile_of_dst) * NPH + phase1
    seg2 = (core_of * T + tile_of_dst) * NPH + phase2
    c1 = np.bincount(seg1, minlength=NC * T * NPH)
    c2 = np.bincount(seg2, minlength=NC * T * NPH)
    assert c1.max() <= SLOTS_PP, f"L1 overflow {c1.max()}"
    assert c2.max() <= SLOTS_PP, f"L2 overflow {c2.max()}"

    # --- per-edge slot assignment (shared helper) ------------------------
    def edge_layout(phase):
        """Returns (CH, p, islot) arrays: meta chunk col, partition, and
        per-(block-call) flat slot index for every edge."""
        seg = (core_of * T + tile_of_dst) * NPH + phase
        rank = _rank_within_groups(seg, NC * T * NPH)
        s = rank                                  # [0, SLOTS_PP)
        ch = tile_of_dst * (NPH * CPP) + phase * CPP + s // P
        p = s % P
        return ch, p, s

    ch1, p1, s1 = edge_layout(phase1)
    ch2, p2, s2 = edge_layout(phase2)

    # table row (chunk-local) per edge
    loc1 = src - phase1 * CHUNK1
    loc2 = h1row_node[src] - phase2 * CHUNK2
    assert loc1.min() >= 0 and loc1.max() < CHUNK1
    assert loc2.min() >= 0 and loc2.max() < CHUNK2

    # --- weights / shared constants --------------------------------------
    W1l = np.asarray(W1l, np.float32); W1r = np.asarray(W1r, np.float32)
    W2l = np.asarray(W2l, np.float32); W2r = np.asarray(W2r, np.float32)
    wts = np.concatenate([W1l.T, W1r.T, W2l.T, W2r.T], axis=1)  # [64, 256]
    bias = np.stack([np.asarray(b1, np.float32),
                     np.asarray(b2, np.float32)], axis=1)       # [64, 2]
    fcw = np.concatenate([np.asarray(fcW, np.float32).T,
                          np.asarray(fcb, np.float32)[None, :]], axis=0)  # [65, 16]

    iota128 = np.tile(np.arange(P, dtype=np.float32)[None, :], (P, 1))
    iota512 = np.tile(np.arange(G, dtype=np.float32)[None, :], (P, 1))

    # x table chunks (shared across cores)
    xpad = np.zeros((NPH * CHUNK1, D), np.float32)
    xpad[:N] = x
    x_chunks = [np.ascontiguousarray(xpad[c * CHUNK1:(c + 1) * CHUNK1])
                for c in range(NPH)]

    # meta column layout
    NCH = T * NPH * CPP          # total chunk count per core per layer
    off_iota128 = 0
    off_iota512 = off_iota128 + P
    off_d1 = off_iota512 + G
    off_r1 = off_d1 + NCH
    off_d2 = off_r1 + NCH
    off_r2 = off_d2 + NCH
    off_gs = off_r2 + NCH
    off_gr = off_gs + T
    CM = off_gr + T

    in_maps = []
    for k in range(NC):
        m = {}
        ek = np.nonzero(core_of == k)[0]
        cmeta = np.zeros((P, CM), np.float32)
        cmeta[:, off_iota128:off_iota128 + P] = iota128
        cmeta[:, off_iota512:off_iota512 + G] = iota512
        dme = np.full((P, NCH), -1.0, np.float32)
        rme = np.zeros((P, NCH), np.float32)
        dme[p1[ek], ch1[ek]] = q_of[dst[ek]].astype(np.float32)
        rme[p1[ek], ch1[ek]] = recip[dst[ek]]
        cmeta[:, off_d1:off_d1 + NCH] = dme
        cmeta[:, off_r1:off_r1 + NCH] = rme
        dme2 = np.full((P, NCH), -1.0, np.float32)
        rme2 = np.zeros((P, NCH), np.float32)
        dme2[p2[ek], ch2[ek]] = q_of[dst[ek]].astype(np.float32)
        rme2[p2[ek], ch2[ek]] = recip[dst[ek]]
        cmeta[:, off_d2:off_d2 + NCH] = dme2
        cmeta[:, off_r2:off_r2 + NCH] = rme2

        # graph slots
        gs = np.full((P, T), -1.0, np.float32)
        gr = np.zeros((P, T), np.float32)
        nodes = np.arange(k * NPC, (k + 1) * NPC)
        gs[q_of[nodes], t_of[nodes]] = batch[nodes].astype(np.float32)
        gr[q_of[nodes], t_of[nodes]] = grecip_g[batch[nodes]]
        cmeta[:, off_gs:off_gs + T] = gs
        cmeta[:, off_gr:off_gr + T] = gr
        m["cmeta"] = cmeta

        # idx tensors: [128, T*128] int16; per (block b, phase c) a call of
        # B*SLOTS_PP idxs -> B*SLOTS_PP/16 cols at offset (b*NPH+c)*B*SLOTS_PP/16
        def build_idx(ch, p, s, loc, phase):
            tile_e = ch // (NPH * CPP)
            flat = np.zeros((NB, NPH, B * SLOTS_PP), np.int64)
            bidx = tile_e // B
            ti = tile_e % B
            islot = ti * SLOTS_PP + s
            flat[bidx, phase, islot] = loc
            cols = []
            for b in range(NB):
                for c in range(NPH):
                    cols.append(_wrap_idx(flat[b, c]))
            return np.concatenate(cols, axis=1)   # [128, NB*NPH*(B*SLOTS_PP/16)]

        m["idxL1"] = build_idx(ch1[ek], p1[ek], s1[ek], loc1[ek], phase1[ek])
        m["idxL2"] = build_idx(ch2[ek], p2[ek], s2[ek], loc2[ek], phase2[ek])

        # xT slab [64, T*128]
        xT = np.zeros((D, T * P), np.float32)
        xT[:, t_of[nodes] * P + q_of[nodes]] = x[nodes].T
        m["xT"] = xT

        m["wts"] = wts
        m["bias"] = bias
        m["fcw"] = fcw
        for c in range(NPH):
            m[f"x{c}"] = x_chunks[c]
        in_maps.append(m)

    info = dict(T=T, NB=NB, CM=CM, NCH=NCH, CHUNK1=CHUNK1, CHUNK2=CHUNK2,
                off=dict(iota128=off_iota128, iota512=off_iota512,
                         d1=off_d1, r1=off_r1, d2=off_d2, r2=off_r2,
                         gs=off_gs, gr=off_gr),
                h1row_node=h1row_node, t_of=t_of, q_of=q_of)
    return in_maps, info


# ---------------------------------------------------------------------------
# Numpy simulation of the device algorithm (for debugging host prep)
# ---------------------------------------------------------------------------

def device_sim(in_maps, info, cfg=CFG):
    N, D, H, O, G = cfg["N"], cfg["D"], cfg["H"], cfg["O"], cfg["G"]
    NC, NPH, CPP, P, B = (cfg["NCORES"], cfg["NPHASE"],
                          cfg["CPP"], cfg["P"], cfg["B"])
    CHUNK2 = info["CHUNK2"]
    SLOTS_PP = CPP * P
    T, NB, off = info["T"], info["NB"], info["off"]
    NCH = info["NCH"]

    def unwrap(idxw, b, c):
        cols = B * SLOTS_PP // 16
        blk = idxw[:16, (b * NPH + c) * cols:(b * NPH + c + 1) * cols]
        return blk.T.reshape(-1).astype(np.int64)   # [B*SLOTS_PP]

    def layer(m, tables, idx_key, offd, offr, rhs_self, wcol):
        hT = np.zeros((H, T * P), np.float32)
        wts = m["wts"]
        for b in range(NB):
            # gather regions: [128, B*16, 64]
            g = np.zeros((P, B * NPH * CPP, D), np.float32)
            for c in range(NPH):
                idxs = unwrap(m[idx_key], b, c)
                rows = tables[c][idxs]              # [B*SLOTS_PP, D]
                for i in range(len(idxs)):
                    g[i % P, c * B * CPP + i // P] = rows[i]
            for ti in range(B):
                t = b * B + ti
                aggrT = np.zeros((D, P), np.float32)
                for c in range(NPH):
                    for j in range(CPP):
                        CH = t * NPH * CPP + c * CPP + j
                        col = c * B * CPP + ti * CPP + j
                        dstrel = m["cmeta"][:, offd + CH]
                        rc = m["cmeta"][:, offr + CH]
                        oh = (m["cmeta"][:, off["iota128"]:off["iota128"] + P]
                              == dstrel[:, None]).astype(np.float32) * rc[:, None]
                        aggrT += g[:, col, :].T @ oh
                pre = (wts[:, wcol:wcol + 64].T @ aggrT
                       + wts[:, wcol + 64:wcol + 128].T @ rhs_self[:, t * P:(t + 1) * P]
                       + m["bias"][:, (0 if wcol == 0 else 1)][:, None])
                hT[:, t * P:(t + 1) * P] = np.maximum(pre, 0.0)
        return hT

    outs = []
    h1T_all = []
    for k in range(NC):
        m = in_maps[k]
        tables1 = [m[f"x{c}"] for c in range(NPH)]
        h1T = layer(m, tables1, "idxL1", off["d1"], off["r1"], m["xT"], 0)
        h1T_all.append(h1T)
    # allgather h1
    h1_full = np.concatenate([h1T.T for h1T in h1T_all], axis=0)  # [8*T*128, 64]
    h1_pad = np.zeros((NPH * CHUNK2, D), np.float32)
    h1_pad[:h1_full.shape[0]] = h1_full
    tables2 = [h1_pad[c * CHUNK2:(c + 1) * CHUNK2] for c in range(NPH)]

    pooled = np.zeros((D, G), np.float32)
    h2T_all = []
    for k in range(NC):
        m = in_maps[k]
        h1T = h1T_all[k]
        h2T = layer(m, tables2, "idxL2", off["d2"], off["r2"], h1T, 128)
        h2T_all.append(h2T)
        for t in range(T):
            gsl = m["cmeta"][:, off["gs"] + t]
            grc = m["cmeta"][:, off["gr"] + t]
            oh = (m["cmeta"][:, off["iota512"]:off["iota512"] + G]
                  == gsl[:, None]).astype(np.float32) * grc[:, None]
            pooled += h2T[:, t * P:(t + 1) * P] @ oh
    # logits
    m0 = in_maps[0]
    poolA = np.concatenate([pooled, np.ones((1, G), np.float32)], axis=0)  # [65, G]
    logits = poolA.T @ m0["fcw"]            # [G, 16]
    mx = logits.max(axis=1, keepdims=True)
    lse = np.log(np.exp(logits - mx).sum(axis=1, keepdims=True))
    return logits - mx - lse


# ---------------------------------------------------------------------------
# Bass kernel builder
# ---------------------------------------------------------------------------

def build_kernel(T, cfg=CFG, repeats=1, skip=()):
    import concourse.bass as bass
    import concourse.bacc as bacc
    import concourse.tile as tile
    import concourse.mybir as mybir
    from concourse.masks import make_identity

    F32 = mybir.dt.float32
    I16 = mybir.dt.int16
    AF = mybir.ActivationFunctionType
    OP = mybir.AluOpType

    N, D, H, O, G = cfg["N"], cfg["D"], cfg["H"], cfg["O"], cfg["G"]
    NC, NPH, CPP, P, B = (cfg["NCORES"], cfg["NPHASE"],
                          cfg["CPP"], cfg["P"], cfg["B"])
    SLOTS_PP = CPP * P
    CHUNK1 = -(-N // NPH)
    CHUNK2 = -(-(NC * T * P) // NPH)
    NB = T // B
    NCH = T * NPH * CPP
    off_iota128 = 0
    off_iota512 = P
    off_d1 = off_iota512 + G
    off_r1 = off_d1 + NCH
    off_d2 = off_r1 + NCH
    off_r2 = off_d2 + NCH
    off_gs = off_r2 + NCH
    off_gr = off_gs + T
    CM = off_gr + T
    IDX_COLS = NB * NPH * (B * SLOTS_PP // 16)
    CALL_COLS = B * SLOTS_PP // 16

    nc = bacc.Bacc("TRN2", target_bir_lowering=False, debug=False,
                   num_devices=NC)

    cmeta_t = nc.dram_tensor("cmeta", [P, CM], F32, kind="ExternalInput")
    idxL1_t = nc.dram_tensor("idxL1", [P, IDX_COLS], I16, kind="ExternalInput")
    idxL2_t = nc.dram_tensor("idxL2", [P, IDX_COLS], I16, kind="ExternalInput")
    xT_t = nc.dram_tensor("xT", [D, T * P], F32, kind="ExternalInput")
    wts_t = nc.dram_tensor("wts", [D, 256], F32, kind="ExternalInput")
    bias_t = nc.dram_tensor("bias", [D, 2], F32, kind="ExternalInput")
    fcw_t = nc.dram_tensor("fcw", [D + 1, O], F32, kind="ExternalInput")
    x_ts = [nc.dram_tensor(f"x{c}", [CHUNK1, D], F32, kind="ExternalInput")
            for c in range(NPH)]
    out_t = nc.dram_tensor("out", [G, O], F32, kind="ExternalOutput")

    H1ROWS = NC * T * P

    with tile.TileContext(nc, num_cores=NC) as tc:
        with (
            tc.tile_pool(name="cst", bufs=1) as cst,
            tc.tile_pool(name="slab", bufs=1) as slab,
            tc.tile_pool(name="idxp", bufs=2) as idxp,
            tc.tile_pool(name="gp", bufs=2) as gp,
            tc.tile_pool(name="ohp", bufs=4) as ohp,
            tc.tile_pool(name="agp", bufs=2) as agp,
            tc.tile_pool(name="rowp", bufs=3) as rowp,
            tc.tile_pool(name="smallp", bufs=2) as smallp,
            tc.tile_pool(name="ps_ag", bufs=2, space="PSUM") as ps_ag,
            tc.tile_pool(name="ps_h", bufs=2, space="PSUM") as ps_h,
            tc.tile_pool(name="ps_tr", bufs=2, space="PSUM") as ps_tr,
            tc.tile_pool(name="ps_pool", bufs=1, space="PSUM") as ps_pool,
            tc.tile_pool(name="dram", bufs=1, space="DRAM") as dram,
        ):
            # ---- constants -------------------------------------------------
            cmeta = cst.tile([P, CM], F32)
            nc.sync.dma_start(cmeta[:], cmeta_t[:])
            wts = cst.tile([D, 256], F32)
            nc.sync.dma_start(wts[:], wts_t[:])
            bias = cst.tile([D, 2], F32)
            nc.sync.dma_start(bias[:], bias_t[:])
            fcw = cst.tile([D + 1, O], F32)
            nc.sync.dma_start(fcw[:], fcw_t[:])
            xT = slab.tile([D, T * P], F32)
            nc.sync.dma_start(xT[:], xT_t[:])
            ident = cst.tile([P, P], F32)
            make_identity(nc, ident[:])

            h1T = slab.tile([D, T * P], F32)

            h1_local = dram.tile([T * P, D], F32)
            pool_in = dram.tile([D, G], F32)
            pool_out = dram.tile([D, G], F32, addr_space="Shared")

            psum_pool = ps_pool.tile([D, G], F32)

            h1_full = None

            def do_layer(lyr):
                idx_t = idxL1_t if lyr == 0 else idxL2_t
                offd = off_d1 if lyr == 0 else off_d2
                offr = off_r1 if lyr == 0 else off_r2
                wcol = 0 if lyr == 0 else 128
                for b in range(NB):
                    idx_sb = idxp.tile([P, NPH * CALL_COLS], I16, tag="idx")
                    nc.sync.dma_start(
                        idx_sb[:],
                        idx_t[:, b * NPH * CALL_COLS:(b + 1) * NPH * CALL_COLS])
                    g = gp.tile([P, B * NPH * CPP, D], F32, tag="g")
                    for c in range(NPH):
                        if lyr == 0:
                            in_ap = x_ts[c][:]
                        else:
                            lo = c * CHUNK2
                            hi = min((c + 1) * CHUNK2, H1ROWS)
                            in_ap = h1_full[:][lo:hi, :]
                        if "gather" in skip:
                            if c == 0:
                                nc.vector.memset(g[:, :, 0:1], 0.0)
                        else:
                            # single_packet=True needs calls <= 1024 idxs;
                            # packed descriptor gen is ~order faster on Q7
                            half = B * SLOTS_PP // 2
                            hc = CALL_COLS // 2
                            hr = B * CPP // 2
                            for hh in range(2):
                                nc.gpsimd.dma_gather(
                                    out_ap=g[:, c * B * CPP + hh * hr:
                                             c * B * CPP + (hh + 1) * hr, :],
                                    in_ap=in_ap,
                                    idxs_ap=idx_sb[:, c * CALL_COLS + hh * hc:
                                                   c * CALL_COLS + (hh + 1) * hc],
                                    num_idxs=half,
                                    num_idxs_reg=half,
                                    elem_size=D,
                                    single_packet=True,
                                )
                    psum_bank = ps_ag.tile([D, B * P], F32, tag="aggr")
                    for ti in range(B):
                        t = b * B + ti
                        nmm = 0
                        for c in range(NPH):
                            for j in range(CPP):
                                CH = t * NPH * CPP + c * CPP + j
                                col = c * B * CPP + ti * CPP + j
                                oh = ohp.tile([P, P], F32, tag="oh")
                                if "onehot" in skip:
                                    continue
                                nc.any.tensor_scalar(
                                    out=oh[:],
                                    in0=cmeta[:, off_iota128:off_iota128 + P],
                                    scalar1=cmeta[:, offd + CH:offd + CH + 1],
                                    scalar2=cmeta[:, offr + CH:offr + CH + 1],
                                    op0=OP.is_equal,
                                    op1=OP.mult,
                                )
                                if "aggrmm" not in skip:
                                    nc.tensor.matmul(
                                        psum_bank[:, ti * P:(ti + 1) * P],
                                        lhsT=g[:, col, :],
                                        rhs=oh[:],
                                        start=(nmm == 0),
                                        stop=(nmm == NPH * CPP - 1),
                                    )
                                nmm += 1
                    aggrT = agp.tile([D, B * P], F32, tag="aggrT")
                    nc.vector.tensor_copy(out=aggrT[:], in_=psum_bank[:])
                    for ti in range(B):
                        t = b * B + ti
                        psum_hT = ps_h.tile([D, P], F32, tag="hT")
                        nc.tensor.matmul(
                            psum_hT[:], lhsT=wts[:, wcol:wcol + 64],
                            rhs=aggrT[:, ti * P:(ti + 1) * P],
                            start=True, stop=False)
                        rhs_self = (xT if lyr == 0 else h1T)
                        nc.tensor.matmul(
                            psum_hT[:], lhsT=wts[:, wcol + 64:wcol + 128],
                            rhs=rhs_self[:, t * P:(t + 1) * P],
                            start=False, stop=True)
                        if lyr == 0:
                            nc.scalar.activation(
                                out=h1T[:, t * P:(t + 1) * P], in_=psum_hT[:],
                                func=AF.Relu, bias=bias[:, 0:1], scale=1.0)
                            psum_tr = ps_tr.tile([P, D], F32, tag="tr")
                            nc.tensor.transpose(
                                psum_tr[:], h1T[:, t * P:(t + 1) * P],
                                ident[:D, :D])
                            h1row = rowp.tile([P, D], F32, tag="row")
                            nc.vector.tensor_copy(out=h1row[:], in_=psum_tr[:])
                            nc.sync.dma_start(
                                h1_local[:][t * P:(t + 1) * P, :], h1row[:])
                        else:
                            h2T = smallp.tile([D, P], F32, tag="h2T")
                            nc.scalar.activation(
                                out=h2T[:], in_=psum_hT[:],
                                func=AF.Relu, bias=bias[:, 1:2], scale=1.0)
                            psum_tr = ps_tr.tile([P, D], F32, tag="tr")
                            nc.tensor.transpose(psum_tr[:], h2T[:],
                                                ident[:D, :D])
                            h2row = rowp.tile([P, D], F32, tag="row")
                            nc.vector.tensor_copy(out=h2row[:], in_=psum_tr[:])
                            ghoh = ohp.tile([P, G], F32, tag="ghoh")
                            nc.any.tensor_scalar(
                                out=ghoh[:],
                                in0=cmeta[:, off_iota512:off_iota512 + G],
                                scalar1=cmeta[:, off_gs + t:off_gs + t + 1],
                                scalar2=cmeta[:, off_gr + t:off_gr + t + 1],
                                op0=OP.is_equal,
                                op1=OP.mult,
                            )
                            nc.tensor.matmul(
                                psum_pool[:], lhsT=h2row[:], rhs=ghoh[:],
                                start=(t == 0), stop=(t == T - 1),
                                skip_group_check=True,
                            )

            for _rep in range(repeats):
                do_layer(0)
                h1_full = dram.tile([H1ROWS, D], F32, addr_space="Shared",
                                    tag="h1f", bufs=repeats)
                nc.gpsimd.collective_compute(
                    "AllGather", mybir.AluOpType.bypass,
                    replica_groups=[list(range(NC))],
                    ins=[h1_local.opt()], outs=[h1_full.opt()],
                )
                do_layer(1)

            # ---- pooled AllReduce + logits + log_softmax -------------------
            pooled_sb = slab.tile([D + 1, G], F32)
            nc.gpsimd.memset(pooled_sb[D:D + 1, :], 1.0)
            nc.vector.tensor_copy(out=pooled_sb[:D, :], in_=psum_pool[:])
            nc.sync.dma_start(pool_in[:], pooled_sb[:D, :])
            nc.gpsimd.collective_compute(
                "AllReduce", mybir.AluOpType.add,
                replica_groups=[list(range(NC))],
                ins=[pool_in.opt()], outs=[pool_out.opt()],
            )
            nc.sync.dma_start(pooled_sb[:D, :], pool_out[:])
            for gt in range(-(-G // P)):
                gsz = min(P, G - gt * P)
                psum_lg = ps_tr.tile([gsz, O], F32, tag="tr")
                nc.tensor.matmul(
                    psum_lg[:], lhsT=pooled_sb[:, gt * P:gt * P + gsz],
                    rhs=fcw[:], start=True, stop=True)
                mx = smallp.tile([gsz, 1], F32, tag="mx")
                nc.vector.tensor_reduce(
                    out=mx[:], in_=psum_lg[:], axis=mybir.AxisListType.X,
                    op=OP.max)
                nmx = smallp.tile([gsz, 1], F32, tag="nmx")
                nc.vector.tensor_scalar(
                    out=nmx[:], in0=mx[:], scalar1=-1.0, scalar2=None,
                    op0=OP.mult)
                ex = smallp.tile([gsz, O], F32, tag="ex")
                sumexp = smallp.tile([gsz, 1], F32, tag="se")
                nc.scalar.activation(
                    out=ex[:], in_=psum_lg[:], func=AF.Exp,
                    bias=nmx[:], scale=1.0, accum_out=sumexp[:])
                lse = smallp.tile([gsz, 1], F32, tag="lse")
                nc.scalar.activation(
                    out=lse[:], in_=sumexp[:], func=AF.Ln)
                res = smallp.tile([gsz, O], F32, tag="res")
                nc.vector.tensor_scalar(
                    out=res[:], in0=psum_lg[:], scalar1=nmx[:],
                    scalar2=lse[:], op0=OP.add, op1=OP.subtract)
                nc.sync.dma_start(out_t[gt * P:gt * P + gsz, :], res[:])

    nc.compile()
    return nc


# ---------------------------------------------------------------------------
# Entry point
# ---------------------------------------------------------------------------

def kernel(x, W1l, b1, W1r, W2l, b2, W2r, fcW, fcb, edge_index, batch,
           _cfg=None, _collect=None):
    cfg = _cfg or CFG
    in_maps, info = preprocess(x, W1l, b1, W1r, W2l, b2, W2r, fcW, fcb,
                               edge_index, batch, cfg)
    key = (info["T"], tuple(sorted(cfg.items())))
    if key not in _BUILD_CACHE:
        _BUILD_CACHE[key] = build_kernel(info["T"], cfg)
    nc = _BUILD_CACHE[key]

    from concourse.bass_utils import run_bass_kernel_spmd
    res = run_bass_kernel_spmd(
        nc, in_maps, core_ids=list(range(cfg["NCORES"])),
        **(_collect or {}))
    if _collect is not None:
        kernel._last_result = res
    return res.results[0]["out"]


if __name__ == "__main__":
    pass
